# revision 55
# baseline (speedup 1.0000x reference)
"""AGCRN cell with per-node MLP-generated gate weights, on 8 TRN2 NeuronCores.

Math (reference):
    combined = adj @ concat([x, h], -1)          # [N, 257]
    cg = combined[nodes_ind]                     # [M, 257]
    gate(f, q, W, b) = einsum('ni,nd,dio->no', f, q, W) + q @ b
    r = sigmoid(gate(cg, q, W_r, b_r)); u = sigmoid(gate(cg, q, W_u, b_u))
    cn = [x_sel, r * h_sel]                      # [M, 257]
    cand = tanh(gate(cn, q, W_c, b_c))
    new_h = (1 - u) * (r * h_sel) + u * cand     # [M, 128]

M rows shard across 8 cores (Mc = 256 rows/core), W_* replicated, no
collectives. Transposed orientation on device: [feat, n].

v2 schedule (trace-driven rework of the 67.4us baseline):
  - Dummy ones-matmul spin at kernel start (no DMA deps) keeps the PE
    busy through the ~10us DMA head so HAM un-throttles to 2.4 GHz
    before real matmuls start (baseline ran cold for 29us).
  - zg is consumed MIXED: 4 of 8 chunks converted bf16->fp8 on ACT and
    matmul'd DoubleRow; 4 consumed bf16 directly against fp8 weights
    (mixed-dtype matmul, validated rel-err 0.0048). This halves the
    baseline's 16.3us of ACT converts and balances PE/ACT/DVE.
  - q_bc psum->SBUF copies split 8 DVE / 8 ACT; all DMA issue stays on
    sync+scalar queues with AT/C first so adj starts ~12us.
  - hTN (h_sel*N bf16) from host: sigmoid->rh_b->zc1 chain is 2 short
    bf16 ops; rh_f/(1-u)rh computed off critical path.
  - c-gate x-part matmuls emitted between r/u and the zc1-paced h-part
    to fill the PE gap under sigmoid/rh/zc1 production.
"""

import os
import sys

sys.path.insert(0, "/opt/trn_rl_repo")

import numpy as np
import ml_dtypes
from ml_dtypes import bfloat16

import concourse.bass as bass
import concourse.tile as tile
from concourse import bacc, mybir
from concourse.bass_utils import run_bass_kernel_spmd

NC = 8
N = 4096
M = 2048
Mc = M // NC  # 256 rows per core
QD = 32
O = 128
S = 128.0  # fp8 scale folded into q_bc / biases; activations apply 1/S
KROWS = QD * 256  # 8192 z rows (d, i<256)
# adj: 32 k-tiles as 2 AT transfers x 16; C0/C1 monolithic
NCONV = 4  # zg chunks converted to fp8 for DoubleRow (rest bf16 mixed)
BF16 = mybir.dt.bfloat16
F8 = mybir.dt.float8e4
F32 = mybir.dt.float32
AF = mybir.ActivationFunctionType
DR = mybir.MatmulPerfMode.DoubleRow
SMW = 3 * 256 + 6 * 128  # packed [32, x] smalls: qT, b_r/u/c, xtail, cgL, Wt_r/u/c

_COMPILED = None


def _dr(ap, half):
    """2D AP slice -> 3D DoubleRow AP [128, 2, half]."""
    return ap.rearrange("p (two m) -> p two m", two=2)


def _build():
    nc = bacc.Bacc("TRN2", target_bir_lowering=False, debug=False, num_devices=NC)
    d_AT = nc.dram_tensor("AT", [128, 32 * Mc], F8, kind="ExternalInput").ap()
    d_C = nc.dram_tensor("C", [128, 64 * 128], F8, kind="ExternalInput").ap()
    d_ZC = nc.dram_tensor("ZC", [128, KROWS], BF16, kind="ExternalInput").ap()
    d_Wr = nc.dram_tensor("Wr", [128, 64 * O], F8, kind="ExternalInput").ap()
    d_Wu = nc.dram_tensor("Wu", [128, 64 * O], F8, kind="ExternalInput").ap()
    d_Wc = nc.dram_tensor("Wc", [128, 64 * O], BF16, kind="ExternalInput").ap()
    d_SM = nc.dram_tensor("SM", [QD, SMW], BF16, kind="ExternalInput").ap()
    d_qTf = nc.dram_tensor("qTflat", [1, KROWS], BF16, kind="ExternalInput").ap()
    d_hTN = nc.dram_tensor("hTN", [128, Mc], BF16, kind="ExternalInput").ap()
    d_out = nc.dram_tensor("out", [O, Mc], F32, kind="ExternalOutput").ap()
    DBG = bool(os.environ.get("BASS_KERNEL_DEBUG"))
    if DBG:
        d_dbg_qbc = nc.dram_tensor("dbg_qbc", [128, KROWS], BF16, kind="ExternalOutput").ap()
        d_dbg_cgT = nc.dram_tensor("dbg_cgT", [128, 2 * Mc], BF16, kind="ExternalOutput").ap()
        d_dbg_r = nc.dram_tensor("dbg_r", [128, Mc], F32, kind="ExternalOutput").ap()
        d_dbg_u = nc.dram_tensor("dbg_u", [128, Mc], F32, kind="ExternalOutput").ap()
        d_dbg_cand = nc.dram_tensor("dbg_cand", [128, Mc], F32, kind="ExternalOutput").ap()

    with tile.TileContext(nc) as tc:
        with (
            tc.tile_pool(name="res", bufs=1) as res,
            tc.tile_pool(name="psum", bufs=1, space=bass.MemorySpace.PSUM) as pp,
        ):
            # --- ACT table preload (sigmoid_and_others: copy/sigmoid/tanh)
            warm = res.tile([1, 8], F32, name="warm")
            nc.vector.memset(warm[:], 0.0)
            warm2 = res.tile([1, 8], F32, name="warm2")
            nc.scalar.activation(warm2[:], warm[:], AF.Sigmoid)
            ones = res.tile([1, 128], BF16, name="ones")
            nc.vector.memset(ones[:], 1.0)
            # warm-up tiles memset on gpsimd (earliest-finishing preamble)
            # so the PE spin starts ~6us, not ~8.3
            wK = res.tile([128, 128], BF16, name="wK")
            nc.gpsimd.memset(wK[:], 0.01)
            wsrc = res.tile([128, 512], BF16, name="wsrc")
            nc.gpsimd.memset(wsrc[:], 0.5)

            # --- DMA. Per-ring bandwidth is only ~135-165 GB/s, so three
            # rings run in parallel (sync + scalar HWDGE, gpsimd SWDGE)
            # and every transfer is placed by its consumption deadline:
            #   sync:   SM, qTf, Ca, Wr0, Wr1, Wch, out
            #   scalar: ATa, Wu0, Wu1, hTN, Wcx
            #   gpsimd: ATb, Cb, zc0
            # Emission order doubles as sem-assignment order: the first 8
            # get fresh sems, later ones reuse sems of long-completed
            # transfers (in-flight reuse stalls desc-gen for ~6us).
            # ONE TILE PER TRANSFER: Tile's dependency tracking is
            # tile-granular, so a reader of a multi-DMA tile falsely waits
            # for ALL of its transfers (cost the baseline ~8us on C alone).
            sm_sb = res.tile([QD, SMW], BF16, name="sm_sb")
            nc.sync.dma_start(sm_sb[:], d_SM[:])
            qTf_sb = res.tile([1, KROWS], BF16, name="qTf_sb")
            nc.sync.dma_start(qTf_sb[:], d_qTf[:])
            AT_sb = res.tile([128, 32 * Mc], F8, name="AT_sb")
            nc.scalar.dma_start(AT_sb[:], d_AT[:])
            C_sb = res.tile([128, 64 * 128], F8, name="C_sb")
            nc.sync.dma_start(C_sb[:], d_C[:])

            Wr_h, Wu_h = [], []
            for ih in range(2):
                sl = slice(ih * 32 * O, (ih + 1) * 32 * O)
                tr = res.tile([128, 32 * O], F8, name=f"Wr{ih}_sb")
                nc.sync.dma_start(tr[:], d_Wr[:, sl])
                Wr_h.append(tr)
                tu = res.tile([128, 32 * O], F8, name=f"Wu{ih}_sb")
                nc.scalar.dma_start(tu[:], d_Wu[:, sl])
                Wu_h.append(tu)
            # gpsimd ring: zc0 half A, Wcx, zc0 half B (deadline order)
            zc0_h = [
                res.tile([128, KROWS // 2], BF16, name=f"zc0{j}_sb") for j in range(2)
            ]
            nc.gpsimd.dma_start(zc0_h[0][:], d_ZC[:, 0 : KROWS // 2])
            Wcx_sb = res.tile([128, 32 * O], BF16, name="Wcx_sb")
            nc.gpsimd.dma_start(Wcx_sb[:], d_Wc[:, 0 : 32 * O])
            nc.gpsimd.dma_start(zc0_h[1][:], d_ZC[:, KROWS // 2 : KROWS])
            hTN_sb = res.tile([128, Mc], BF16, name="hTN_sb")
            nc.scalar.dma_start(hTN_sb[:], d_hTN[:])
            Wch_sb = res.tile([128, 32 * O], BF16, name="Wch_sb")
            nc.scalar.dma_start(Wch_sb[:], d_Wc[:, 32 * O : 64 * O])

            qT_sb = sm_sb[:, 0:Mc]
            b_sb = {g: sm_sb[:, Mc + i * O : Mc + (i + 1) * O] for i, g in enumerate("ruc")}
            xtail_sb = sm_sb[:, Mc + 3 * O : 2 * Mc + 3 * O]
            cgL_sb = sm_sb[:, 2 * Mc + 3 * O : 3 * Mc + 3 * O]
            Wt_sb = {
                g: sm_sb[:, 3 * Mc + (3 + i) * O : 3 * Mc + (4 + i) * O]
                for i, g in enumerate("ruc")
            }

            # --- PSUM. qp: 2 rotating [128,512] banks for the q broadcast;
            # pdum: dedicated bank for warm-up/filler matmuls (never read,
            # so fillers can't stall on a pending copy); pr/pu/pc/pcg get
            # their own banks. Total 8.
            qp = [pp.tile([128, 512], F32, name=f"qp{i}") for i in range(2)]
            pdum = pp.tile([128, 512], F32, name="pdum")[:]
            pr = pp.tile([128, Mc], F32, name="pr")[:]
            pu = pp.tile([128, Mc], F32, name="pu")[:]
            pc = pp.tile([128, Mc], F32, name="pc")[:]
            pcg = [pp.tile([128, Mc], F32, name=f"pcg{i}")[:] for i in range(2)]

            # q_bc as 4 per-k-chunk tiles: zg chunk k then only waits its
            # own 4 copies (tile-granular dependency tracking)
            q_bck = [res.tile([128, 2048], BF16, name=f"q_bc{k}") for k in range(4)]

            def dummy_mm(n):
                # full-K HAM-keepalive matmuls; no deps, no readers
                for _ in range(n):
                    nc.tensor.matmul(pdum, wK[:], wsrc[:], start=True, stop=True)

            # --- PE warm-up spin: no DMA deps, keeps the array busy from
            # ~6us so HAM un-throttles before real work arrives. Full-K
            # matmuls (k=1 broadcasts don't register as HAM activity).
            dummy_mm(10)

            def qbc_mm(i):
                nc.tensor.matmul(
                    qp[i % 2][:],
                    ones[:],
                    qTf_sb[:, i * 512 : (i + 1) * 512],
                    start=True,
                    stop=True,
                )

            def qbc_copy(i):
                # psum -> SBUF; parity split DVE/ACT so consecutive copies
                # run on different engines (MM cadence ~0.33us)
                dst = q_bck[i // 4][:, (i % 4) * 512 : (i % 4 + 1) * 512]
                if i % 2 == 0:
                    nc.vector.tensor_copy(dst, qp[i % 2][:])
                else:
                    nc.scalar.activation(dst, qp[i % 2][:], AF.Copy)

            def adj_group(gi):
                # all 16 k-tile DR pairs for one column-group
                for t in range(16):
                    atap = _dr(AT_sb[:, t * 2 * Mc : (t + 1) * 2 * Mc], Mc)
                    nc.tensor.matmul(
                        pcg[gi],
                        _dr(C_sb[:, gi * 4096 + t * 256 : gi * 4096 + (t + 1) * 256], 128),
                        atap,
                        start=(t == 0),
                        stop=(t == 15),
                        perf_mode=DR,
                    )

            # --- PE: bias matmuls + q_bc broadcasts + adj groups
            nc.tensor.matmul(pr, b_sb["r"], qT_sb, start=True, stop=False)
            nc.tensor.matmul(pu, b_sb["u"], qT_sb, start=True, stop=False)
            nc.tensor.matmul(pc, b_sb["c"], qT_sb, start=True, stop=False)
            for i in range(8):
                qbc_mm(i)
                qbc_copy(i)
            adj_group(0)
            for i in range(8, 16):
                qbc_mm(i)
                qbc_copy(i)
            adj_group(1)

            # --- zgt/zct tails on DVE; zc0 chunks (x-part of c) on the
            # otherwise-idle GpSimd so they don't delay the zg pipeline.
            def zchunk(dst_ap, src_ap, k, eng=None):
                # dst[p, dd*Mc + n] = src[p, n] * q_bc_k[p, dd*Mc + n]
                (eng or nc.vector).tensor_mul(
                    dst_ap.rearrange("p (a b) -> p a b", b=Mc),
                    src_ap.unsqueeze(1).broadcast_to((128, 8, Mc)),
                    q_bck[k][:].rearrange("p (a b) -> p a b", b=Mc),
                )

            zgt = res.tile([QD, Mc], BF16, name="zgt")
            nc.vector.tensor_mul(zgt[:], qT_sb, cgL_sb)
            zct = res.tile([QD, Mc], BF16, name="zct")
            nc.vector.tensor_mul(zct[:], qT_sb, xtail_sb)

            cgT = [res.tile([128, Mc], BF16, name=f"cgT{i}") for i in range(2)]
            nc.vector.tensor_copy(cgT[0][:], pcg[0])

            # --- zg production (DVE, bf16) + r/u gate matmuls.
            # Chunk order (ih, k): ih0 chunks flow as soon as cgT[0] lands
            # (C0 adj group finishes first). The last NCONV chunks (ih1)
            # are converted to fp8 (split ACT / DVE) and consumed
            # DoubleRow; the rest are consumed bf16 against fp8 weights.
            zgb = [res.tile([128, 2048], BF16, name=f"zgb{i}") for i in range(3)]
            zg8 = [res.tile([128, 2048], F8, name=f"zg8_{i}") for i in range(NCONV)]
            ci = 0
            nconv = 0
            for ih in range(2):
                if ih == 1:
                    nc.vector.tensor_copy(cgT[1][:], pcg[1])
                for k in range(4):
                    base = k * 1024  # within the ih-half weight tile
                    Wpair = ((Wr_h[ih], pr), (Wu_h[ih], pu))
                    if ci >= 8 - NCONV:
                        cvi = nconv
                        nconv += 1
                        stg = zgb[ci % 3]
                        zchunk(stg[:], cgT[ih][:], k)
                        # converts split: first half ACT, second half DVE
                        if cvi < NCONV // 2:
                            nc.scalar.activation(zg8[cvi][:], stg[:], AF.Copy)
                        else:
                            nc.vector.tensor_copy(zg8[cvi][:], stg[:])
                        for e in range(4):
                            zap = _dr(zg8[cvi][:, e * 512 : (e + 1) * 512], Mc)
                            for W_sb, pg in Wpair:
                                nc.tensor.matmul(
                                    pg,
                                    _dr(W_sb[:, base + e * 256 : base + (e + 1) * 256], O),
                                    zap,
                                    start=False,
                                    stop=False,
                                    perf_mode=DR,
                                )
                    else:
                        stg = zgb[ci % 3]  # bf16 chunks consumed in place
                        zchunk(stg[:], cgT[ih][:], k)
                        for dd in range(8):
                            zap = stg[:, dd * Mc : (dd + 1) * Mc]
                            for W_sb, pg in Wpair:
                                nc.tensor.matmul(
                                    pg,
                                    W_sb[:, base + dd * O : base + (dd + 1) * O],
                                    zap,
                                    start=False,
                                    stop=False,
                                )
                    ci += 1
            nc.tensor.matmul(pr, Wt_sb["r"], zgt[:], start=False, stop=True)
            nc.tensor.matmul(pu, Wt_sb["u"], zgt[:], start=False, stop=True)

            r_sb = res.tile([128, Mc], BF16, name="r_sb")
            nc.scalar.activation(r_sb[:], pr, AF.Sigmoid, scale=1.0 / S)
            u_sb = res.tile([128, Mc], F32, name="u_sb")
            nc.scalar.activation(u_sb[:], pu, AF.Sigmoid, scale=1.0 / S)

            # --- PE: c-gate x-part (host-made zc0) fills the gap while
            # sigmoid/rh/zc1 run; half A (d 0-15) lands first
            for d in range(32):
                nc.tensor.matmul(
                    pc,
                    Wcx_sb[:, d * O : (d + 1) * O],
                    zc0_h[d // 16][:, (d % 16) * Mc : (d % 16 + 1) * Mc],
                    start=False,
                    stop=False,
                )
            nc.tensor.matmul(pc, Wt_sb["c"], zct[:], start=False, stop=False)

            # --- tail: rh_b (bf16, short chain) -> zc1 chunks + c h-part
            rh_b = res.tile([128, Mc], BF16, name="rh_b")
            nc.vector.tensor_mul(rh_b[:], r_sb[:], hTN_sb[:])
            zc1 = [res.tile([128, 2048], BF16, name=f"zc1_{k}") for k in range(4)]
            for k in range(4):
                zchunk(zc1[k][:], rh_b[:], k)
                for dd in range(8):
                    d = k * 8 + dd
                    nc.tensor.matmul(
                        pc,
                        Wch_sb[:, d * O : (d + 1) * O],
                        zc1[k][:, dd * Mc : (dd + 1) * Mc],
                        start=False,
                        stop=(d == 31),
                    )
            # off-critical: rh_f fp32 derived from rh_b (saves the hT DMA)
            rh_f = res.tile([128, Mc], F32, name="rh_f")
            nc.vector.tensor_scalar_mul(rh_f[:], rh_b[:], 1.0 / float(N))
            t1 = res.tile([128, Mc], F32, name="t1")
            nc.vector.tensor_mul(t1[:], u_sb[:], rh_f[:])
            rh1u = res.tile([128, Mc], F32, name="rh1u")
            nc.vector.tensor_sub(rh1u[:], rh_f[:], t1[:])
            cand_sb = res.tile([128, Mc], F32, name="cand_sb")
            nc.scalar.activation(cand_sb[:], pc, AF.Tanh, scale=1.0 / S)

            t2 = res.tile([128, Mc], F32, name="t2")
            nc.vector.tensor_mul(t2[:], u_sb[:], cand_sb[:])
            outT = res.tile([128, Mc], F32, name="outT")
            nc.vector.tensor_add(outT[:], rh1u[:], t2[:])
            nc.sync.dma_start(d_out[:], outT[:])
            if DBG:
                for k in range(4):
                    nc.sync.dma_start(d_dbg_qbc[:, k * 2048 : (k + 1) * 2048], q_bck[k][:])
                nc.sync.dma_start(d_dbg_cgT[:, 0:Mc], cgT[0][:])
                nc.sync.dma_start(d_dbg_cgT[:, Mc : 2 * Mc], cgT[1][:])
                nc.sync.dma_start(d_dbg_r[:], rh_b[:])
                nc.sync.dma_start(d_dbg_u[:], u_sb[:])
                nc.sync.dma_start(d_dbg_cand[:], cand_sb[:])

    nc.compile()
    return nc


def _get_compiled():
    global _COMPILED
    if _COMPILED is None:
        _COMPILED = _build()
    return _COMPILED


def _pmajor(a, width):
    """[n_tiles*128, width] row-major -> [128, n_tiles*width] partition-major."""
    nt = a.shape[0] // 128
    return np.ascontiguousarray(
        a.reshape(nt, 128, width).transpose(1, 0, 2).reshape(128, nt * width)
    )


def _prep_inputs(x, h, query_vectors, adj, nodes_ind, W_u, b_u, W_r, b_r, W_c, b_c):
    idx = np.asarray(nodes_ind).astype(np.int64)
    f32 = np.float32
    f8 = ml_dtypes.float8_e4m3
    x = np.asarray(x, f32)
    h = np.asarray(h, f32)
    q = np.asarray(query_vectors, f32)
    adj = np.asarray(adj, f32)

    C_dev = np.concatenate([x[:, :128], h], axis=1)  # [N, 256]
    C_pm = np.concatenate(
        [_pmajor(C_dev[:, :128], 128), _pmajor(C_dev[:, 128:], 128)], axis=1
    ).astype(f8)  # [128, 64*128]: C0 block then C1 block
    A_sel = adj[idx]  # [M, N]
    x_sel = x[idx]
    h_sel = h[idx]
    cg_L = A_sel @ x[:, 128]  # [M] leftover mixed feature, on host

    def flat_main(W):
        W = np.asarray(W, f32)
        main = np.concatenate([W[:, :128, :], W[:, 129:, :]], axis=1).reshape(
            QD * 256, O
        )
        return main.reshape(QD * 2, 128, O)  # tile index = d*2 + ihalf

    def order_ru(tiles):
        # (ih, k, d') order — ih-major so the ih0 half is one contiguous
        # DMA that lands before the first bf16 gate matmuls; within a
        # (k, ih) block the 8 d-tiles are consecutive so DR pairs and
        # bf16 singles both slice cleanly
        order = []
        for ih in range(2):
            for k in range(4):
                for dd in range(8):
                    d = 8 * k + dd
                    order.append(d * 2 + ih)
        return tiles[order].reshape(QD * 256, O)

    def order_c(tiles):
        # ihalf-major: all (d, 0) tiles then all (d, 1)
        return np.concatenate([tiles[0::2], tiles[1::2]], axis=0).reshape(
            QD * 256, O
        )

    Wr_pm = _pmajor(order_ru(flat_main(W_r)), O).astype(f8)
    Wu_pm = _pmajor(order_ru(flat_main(W_u)), O).astype(f8)
    Wc_pm = _pmajor(order_c(flat_main(W_c)), O).astype(bfloat16)
    Wt = {g: np.ascontiguousarray(np.asarray(W, f32)[:, 128, :]).astype(bfloat16)
          for g, W in (("r", W_r), ("u", W_u), ("c", W_c))}
    bf_ = {g: (np.asarray(b, f32) * S).astype(bfloat16)
           for g, b in (("r", b_r), ("u", b_u), ("c", b_c))}

    in_maps = []
    for c in range(NC):
        sl = slice(c * Mc, (c + 1) * Mc)
        qT = np.ascontiguousarray(q[sl].T).astype(bfloat16)  # [32, 256]
        AT = np.ascontiguousarray(A_sel[sl].T * np.float32(N))  # [4096, 256]
        SM = np.concatenate(
            [
                qT,
                bf_["r"],
                bf_["u"],
                bf_["c"],
                np.broadcast_to(x_sel[sl, 128] * S, (QD, Mc)).astype(bfloat16),
                np.broadcast_to(cg_L[sl] * S, (QD, Mc)).astype(bfloat16),
                Wt["r"],
                Wt["u"],
                Wt["c"],
            ],
            axis=1,
        )
        # zc0 host-side: ZC[i, d*Mc + n] = x_sel[n, i] * q[n, d] * S
        ZC = (
            x_sel[sl, :128].T[:, None, :] * (q[sl].T[None, :, :] * np.float32(S))
        ).reshape(128, KROWS)
        in_maps.append(
            {
                "AT": _pmajor(AT, Mc).astype(f8),
                "C": C_pm,
                "ZC": ZC.astype(bfloat16),
                "Wr": Wr_pm,
                "Wu": Wu_pm,
                "Wc": Wc_pm,
                "SM": np.ascontiguousarray(SM),
                "qTflat": (qT.reshape(1, KROWS).astype(f32) * (S / N)).astype(bfloat16),
                "hTN": np.ascontiguousarray(h_sel[sl].T * np.float32(N)).astype(bfloat16),
            }
        )
    return in_maps


def run(inputs: dict, trace: bool = False):
    nc = _get_compiled()
    in_maps = _prep_inputs(**inputs)
    res = run_bass_kernel_spmd(nc, in_maps, core_ids=list(range(NC)), trace=trace)
    shards = [res.results[c]["out"].T for c in range(NC)]  # each [256, 128]
    out = np.concatenate(shards, axis=0).astype(np.float32)  # [M, 128]
    return out, res


def kernel(**inputs) -> np.ndarray:
    out, _ = run(inputs, trace=bool(os.environ.get("BASS_KERNEL_TRACE")))
    return out


# revision 58
# speedup vs baseline: 1.0369x; 1.0369x over previous
"""AGCRN cell with per-node MLP-generated gate weights, on 8 TRN2 NeuronCores.

Math (reference):
    combined = adj @ concat([x, h], -1)          # [N, 257]
    cg = combined[nodes_ind]                     # [M, 257]
    gate(f, q, W, b) = einsum('ni,nd,dio->no', f, q, W) + q @ b
    r = sigmoid(gate(cg, q, W_r, b_r)); u = sigmoid(gate(cg, q, W_u, b_u))
    cn = [x_sel, r * h_sel]                      # [M, 257]
    cand = tanh(gate(cn, q, W_c, b_c))
    new_h = (1 - u) * (r * h_sel) + u * cand     # [M, 128]

M rows shard across 8 cores (Mc = 256 rows/core), W_* replicated, no
collectives. Transposed orientation on device: [feat, n].

v2 schedule (trace-driven rework of the 67.4us baseline):
  - Dummy ones-matmul spin at kernel start (no DMA deps) keeps the PE
    busy through the ~10us DMA head so HAM un-throttles to 2.4 GHz
    before real matmuls start (baseline ran cold for 29us).
  - zg is consumed MIXED: 4 of 8 chunks converted bf16->fp8 on ACT and
    matmul'd DoubleRow; 4 consumed bf16 directly against fp8 weights
    (mixed-dtype matmul, validated rel-err 0.0048). This halves the
    baseline's 16.3us of ACT converts and balances PE/ACT/DVE.
  - q_bc psum->SBUF copies split 8 DVE / 8 ACT; all DMA issue stays on
    sync+scalar queues with AT/C first so adj starts ~12us.
  - hTN (h_sel*N bf16) from host: sigmoid->rh_b->zc1 chain is 2 short
    bf16 ops; rh_f/(1-u)rh computed off critical path.
  - c-gate x-part matmuls emitted between r/u and the zc1-paced h-part
    to fill the PE gap under sigmoid/rh/zc1 production.
"""

import os
import sys

sys.path.insert(0, "/opt/trn_rl_repo")

import numpy as np
import ml_dtypes
from ml_dtypes import bfloat16

import concourse.bass as bass
import concourse.tile as tile
from concourse import bacc, mybir
from concourse.bass_utils import run_bass_kernel_spmd

NC = 8
N = 4096
M = 2048
Mc = M // NC  # 256 rows per core
QD = 32
O = 128
S = 128.0  # fp8 scale folded into q_bc / biases; activations apply 1/S
KROWS = QD * 256  # 8192 z rows (d, i<256)
# adj: 32 k-tiles as 2 AT transfers x 16; C0/C1 monolithic
NCONV = 4  # zg chunks converted to fp8 for DoubleRow (rest bf16 mixed)
BF16 = mybir.dt.bfloat16
F8 = mybir.dt.float8e4
F32 = mybir.dt.float32
AF = mybir.ActivationFunctionType
DR = mybir.MatmulPerfMode.DoubleRow
SMW = 3 * 256 + 6 * 128  # packed [32, x] smalls: qT, b_r/u/c, xtail, cgL, Wt_r/u/c

_COMPILED = None


def _dr(ap, half):
    """2D AP slice -> 3D DoubleRow AP [128, 2, half]."""
    return ap.rearrange("p (two m) -> p two m", two=2)


def _build():
    nc = bacc.Bacc("TRN2", target_bir_lowering=False, debug=False, num_devices=NC)
    d_AT = nc.dram_tensor("AT", [128, 32 * Mc], F8, kind="ExternalInput").ap()
    d_C = nc.dram_tensor("C", [128, 64 * 128], F8, kind="ExternalInput").ap()
    d_ZC = nc.dram_tensor("ZC", [128, KROWS], BF16, kind="ExternalInput").ap()
    d_Wr = nc.dram_tensor("Wr", [128, 64 * O], F8, kind="ExternalInput").ap()
    d_Wu = nc.dram_tensor("Wu", [128, 64 * O], F8, kind="ExternalInput").ap()
    d_Wc = nc.dram_tensor("Wc", [128, 64 * O], BF16, kind="ExternalInput").ap()
    d_SM = nc.dram_tensor("SM", [QD, SMW], BF16, kind="ExternalInput").ap()
    d_qTf = nc.dram_tensor("qTflat", [1, KROWS], BF16, kind="ExternalInput").ap()
    d_hTN = nc.dram_tensor("hTN", [128, Mc], BF16, kind="ExternalInput").ap()
    d_out = nc.dram_tensor("out", [O, Mc], F32, kind="ExternalOutput").ap()
    DBG = bool(os.environ.get("BASS_KERNEL_DEBUG"))
    if DBG:
        d_dbg_qbc = nc.dram_tensor("dbg_qbc", [128, KROWS], BF16, kind="ExternalOutput").ap()
        d_dbg_cgT = nc.dram_tensor("dbg_cgT", [128, 2 * Mc], BF16, kind="ExternalOutput").ap()
        d_dbg_r = nc.dram_tensor("dbg_r", [128, Mc], F32, kind="ExternalOutput").ap()
        d_dbg_u = nc.dram_tensor("dbg_u", [128, Mc], F32, kind="ExternalOutput").ap()
        d_dbg_cand = nc.dram_tensor("dbg_cand", [128, Mc], F32, kind="ExternalOutput").ap()

    with tile.TileContext(nc) as tc:
        with (
            tc.tile_pool(name="res", bufs=1) as res,
            tc.tile_pool(name="psum", bufs=1, space=bass.MemorySpace.PSUM) as pp,
        ):
            # --- ACT table preload (sigmoid_and_others: copy/sigmoid/tanh)
            warm = res.tile([1, 8], F32, name="warm")
            nc.vector.memset(warm[:], 0.0)
            warm2 = res.tile([1, 8], F32, name="warm2")
            nc.scalar.activation(warm2[:], warm[:], AF.Sigmoid)
            ones = res.tile([1, 128], BF16, name="ones")
            nc.vector.memset(ones[:], 1.0)
            # warm-up tiles memset on gpsimd (earliest-finishing preamble)
            # so the PE spin starts ~6us, not ~8.3
            wK = res.tile([128, 128], BF16, name="wK")
            nc.gpsimd.memset(wK[:], 0.01)
            wsrc = res.tile([128, 512], BF16, name="wsrc")
            nc.gpsimd.memset(wsrc[:], 0.5)

            # --- DMA. Per-ring bandwidth is only ~135-165 GB/s, so three
            # rings run in parallel (sync + scalar HWDGE, gpsimd SWDGE)
            # and every transfer is placed by its consumption deadline:
            #   sync:   SM, qTf, Ca, Wr0, Wr1, Wch, out
            #   scalar: ATa, Wu0, Wu1, hTN, Wcx
            #   gpsimd: ATb, Cb, zc0
            # Emission order doubles as sem-assignment order: the first 8
            # get fresh sems, later ones reuse sems of long-completed
            # transfers (in-flight reuse stalls desc-gen for ~6us).
            # ONE TILE PER TRANSFER: Tile's dependency tracking is
            # tile-granular, so a reader of a multi-DMA tile falsely waits
            # for ALL of its transfers (cost the baseline ~8us on C alone).
            sm_sb = res.tile([QD, SMW], BF16, name="sm_sb")
            nc.sync.dma_start(sm_sb[:], d_SM[:])
            qTf_sb = res.tile([1, KROWS], BF16, name="qTf_sb")
            nc.sync.dma_start(qTf_sb[:], d_qTf[:])
            AT_sb = res.tile([128, 32 * Mc], F8, name="AT_sb")
            nc.scalar.dma_start(AT_sb[:], d_AT[:])
            C_sb = res.tile([128, 64 * 128], F8, name="C_sb")
            nc.sync.dma_start(C_sb[:], d_C[:])

            Wr_sb = res.tile([128, 64 * O], F8, name="Wr_sb")
            nc.sync.dma_start(Wr_sb[:], d_Wr[:])
            Wu_sb = res.tile([128, 64 * O], F8, name="Wu_sb")
            nc.scalar.dma_start(Wu_sb[:], d_Wu[:])
            # gpsimd ring: zc0 half A, Wcx, zc0 half B (deadline order)
            zc0_h = [
                res.tile([128, KROWS // 2], BF16, name=f"zc0{j}_sb") for j in range(2)
            ]
            nc.gpsimd.dma_start(zc0_h[0][:], d_ZC[:, 0 : KROWS // 2])
            Wcx_sb = res.tile([128, 32 * O], BF16, name="Wcx_sb")
            nc.gpsimd.dma_start(Wcx_sb[:], d_Wc[:, 0 : 32 * O])
            nc.gpsimd.dma_start(zc0_h[1][:], d_ZC[:, KROWS // 2 : KROWS])
            hTN_sb = res.tile([128, Mc], BF16, name="hTN_sb")
            nc.scalar.dma_start(hTN_sb[:], d_hTN[:])
            Wch_sb = res.tile([128, 32 * O], BF16, name="Wch_sb")
            nc.scalar.dma_start(Wch_sb[:], d_Wc[:, 32 * O : 64 * O])

            qT_sb = sm_sb[:, 0:Mc]
            b_sb = {g: sm_sb[:, Mc + i * O : Mc + (i + 1) * O] for i, g in enumerate("ruc")}
            xtail_sb = sm_sb[:, Mc + 3 * O : 2 * Mc + 3 * O]
            cgL_sb = sm_sb[:, 2 * Mc + 3 * O : 3 * Mc + 3 * O]
            Wt_sb = {
                g: sm_sb[:, 3 * Mc + (3 + i) * O : 3 * Mc + (4 + i) * O]
                for i, g in enumerate("ruc")
            }

            # --- PSUM. qp: 2 rotating [128,512] banks for the q broadcast;
            # pdum: dedicated bank for warm-up/filler matmuls (never read,
            # so fillers can't stall on a pending copy); pr/pu/pc/pcg get
            # their own banks. Total 8.
            qp = [pp.tile([128, 512], F32, name=f"qp{i}") for i in range(2)]
            pdum = pp.tile([128, 512], F32, name="pdum")[:]
            pr = pp.tile([128, Mc], F32, name="pr")[:]
            pu = pp.tile([128, Mc], F32, name="pu")[:]
            pc = pp.tile([128, Mc], F32, name="pc")[:]
            pcg = [pp.tile([128, Mc], F32, name=f"pcg{i}")[:] for i in range(2)]

            # q_bc as 4 per-k-chunk tiles: zg chunk k then only waits its
            # own 4 copies (tile-granular dependency tracking)
            q_bck = [res.tile([128, 2048], BF16, name=f"q_bc{k}") for k in range(4)]

            def dummy_mm(n):
                # full-K HAM-keepalive matmuls; no deps, no readers
                for _ in range(n):
                    nc.tensor.matmul(pdum, wK[:], wsrc[:], start=True, stop=True)

            # --- PE warm-up spin: no DMA deps, keeps the array busy from
            # ~6us so HAM un-throttles before real work arrives. Full-K
            # matmuls (k=1 broadcasts don't register as HAM activity).
            dummy_mm(10)

            def qbc_mm(i):
                nc.tensor.matmul(
                    qp[i % 2][:],
                    ones[:],
                    qTf_sb[:, i * 512 : (i + 1) * 512],
                    start=True,
                    stop=True,
                )

            def qbc_copy(i):
                # psum -> SBUF; parity split DVE/ACT so consecutive copies
                # run on different engines (MM cadence ~0.33us)
                dst = q_bck[i // 4][:, (i % 4) * 512 : (i % 4 + 1) * 512]
                if i % 2 == 0:
                    nc.vector.tensor_copy(dst, qp[i % 2][:])
                else:
                    nc.scalar.activation(dst, qp[i % 2][:], AF.Copy)

            def adj_group(gi):
                # all 16 k-tile DR pairs for one column-group
                for t in range(16):
                    atap = _dr(AT_sb[:, t * 2 * Mc : (t + 1) * 2 * Mc], Mc)
                    nc.tensor.matmul(
                        pcg[gi],
                        _dr(C_sb[:, gi * 4096 + t * 256 : gi * 4096 + (t + 1) * 256], 128),
                        atap,
                        start=(t == 0),
                        stop=(t == 15),
                        perf_mode=DR,
                    )

            # --- PE: bias matmuls + q_bc broadcasts + adj groups
            nc.tensor.matmul(pr, b_sb["r"], qT_sb, start=True, stop=False)
            nc.tensor.matmul(pu, b_sb["u"], qT_sb, start=True, stop=False)
            nc.tensor.matmul(pc, b_sb["c"], qT_sb, start=True, stop=False)
            for i in range(8):
                qbc_mm(i)
                qbc_copy(i)
            adj_group(0)
            for i in range(8, 16):
                qbc_mm(i)
                qbc_copy(i)
            adj_group(1)

            # --- zgt/zct tails on DVE; zc0 chunks (x-part of c) on the
            # otherwise-idle GpSimd so they don't delay the zg pipeline.
            def zchunk(dst_ap, src_ap, k, eng=None):
                # dst[p, dd*Mc + n] = src[p, n] * q_bc_k[p, dd*Mc + n]
                (eng or nc.vector).tensor_mul(
                    dst_ap.rearrange("p (a b) -> p a b", b=Mc),
                    src_ap.unsqueeze(1).broadcast_to((128, 8, Mc)),
                    q_bck[k][:].rearrange("p (a b) -> p a b", b=Mc),
                )

            zgt = res.tile([QD, Mc], BF16, name="zgt")
            nc.vector.tensor_mul(zgt[:], qT_sb, cgL_sb)
            zct = res.tile([QD, Mc], BF16, name="zct")
            nc.vector.tensor_mul(zct[:], qT_sb, xtail_sb)

            cgT = [res.tile([128, Mc], BF16, name=f"cgT{i}") for i in range(2)]
            nc.vector.tensor_copy(cgT[0][:], pcg[0])

            # --- zg production (DVE, bf16) + r/u gate matmuls.
            # Chunk order (ih, k): ih0 chunks flow as soon as cgT[0] lands
            # (C0 adj group finishes first). The last NCONV chunks (ih1)
            # are converted to fp8 (split ACT / DVE) and consumed
            # DoubleRow; the rest are consumed bf16 against fp8 weights.
            zgb = [res.tile([128, 2048], BF16, name=f"zgb{i}") for i in range(3)]
            zg8 = [res.tile([128, 2048], F8, name=f"zg8_{i}") for i in range(NCONV)]
            ci = 0
            nconv = 0
            for ih in range(2):
                if ih == 1:
                    nc.vector.tensor_copy(cgT[1][:], pcg[1])
                for k in range(4):
                    base = (k * 2 + ih) * 1024
                    Wpair = ((Wr_sb, pr), (Wu_sb, pu))
                    if ci >= 8 - NCONV:
                        cvi = nconv
                        nconv += 1
                        stg = zgb[ci % 3]
                        zchunk(stg[:], cgT[ih][:], k)
                        # converts split: first half ACT, second half DVE
                        if cvi < NCONV // 2:
                            nc.scalar.activation(zg8[cvi][:], stg[:], AF.Copy)
                        else:
                            nc.vector.tensor_copy(zg8[cvi][:], stg[:])
                        for e in range(4):
                            zap = _dr(zg8[cvi][:, e * 512 : (e + 1) * 512], Mc)
                            for W_sb, pg in Wpair:
                                nc.tensor.matmul(
                                    pg,
                                    _dr(W_sb[:, base + e * 256 : base + (e + 1) * 256], O),
                                    zap,
                                    start=False,
                                    stop=False,
                                    perf_mode=DR,
                                )
                    else:
                        stg = zgb[ci % 3]  # bf16 chunks consumed in place
                        zchunk(stg[:], cgT[ih][:], k)
                        for dd in range(8):
                            zap = stg[:, dd * Mc : (dd + 1) * Mc]
                            for W_sb, pg in Wpair:
                                nc.tensor.matmul(
                                    pg,
                                    W_sb[:, base + dd * O : base + (dd + 1) * O],
                                    zap,
                                    start=False,
                                    stop=False,
                                )
                    ci += 1
            nc.tensor.matmul(pr, Wt_sb["r"], zgt[:], start=False, stop=True)
            nc.tensor.matmul(pu, Wt_sb["u"], zgt[:], start=False, stop=True)

            r_sb = res.tile([128, Mc], BF16, name="r_sb")
            nc.scalar.activation(r_sb[:], pr, AF.Sigmoid, scale=1.0 / S)
            u_sb = res.tile([128, Mc], F32, name="u_sb")
            nc.scalar.activation(u_sb[:], pu, AF.Sigmoid, scale=1.0 / S)

            # --- PE: c-gate x-part (host-made zc0) fills the gap while
            # sigmoid/rh/zc1 run; half A (d 0-15) lands first
            for d in range(32):
                nc.tensor.matmul(
                    pc,
                    Wcx_sb[:, d * O : (d + 1) * O],
                    zc0_h[d // 16][:, (d % 16) * Mc : (d % 16 + 1) * Mc],
                    start=False,
                    stop=False,
                )
            nc.tensor.matmul(pc, Wt_sb["c"], zct[:], start=False, stop=False)

            # --- tail: rh_b (bf16, short chain) -> zc1 chunks + c h-part
            rh_b = res.tile([128, Mc], BF16, name="rh_b")
            nc.vector.tensor_mul(rh_b[:], r_sb[:], hTN_sb[:])
            zc1 = [res.tile([128, 2048], BF16, name=f"zc1_{k}") for k in range(4)]
            for k in range(4):
                zchunk(zc1[k][:], rh_b[:], k)
                for dd in range(8):
                    d = k * 8 + dd
                    nc.tensor.matmul(
                        pc,
                        Wch_sb[:, d * O : (d + 1) * O],
                        zc1[k][:, dd * Mc : (dd + 1) * Mc],
                        start=False,
                        stop=(d == 31),
                    )
            # off-critical: rh_f fp32 derived from rh_b (saves the hT DMA)
            rh_f = res.tile([128, Mc], F32, name="rh_f")
            nc.vector.tensor_scalar_mul(rh_f[:], rh_b[:], 1.0 / float(N))
            t1 = res.tile([128, Mc], F32, name="t1")
            nc.vector.tensor_mul(t1[:], u_sb[:], rh_f[:])
            rh1u = res.tile([128, Mc], F32, name="rh1u")
            nc.vector.tensor_sub(rh1u[:], rh_f[:], t1[:])
            cand_sb = res.tile([128, Mc], F32, name="cand_sb")
            nc.scalar.activation(cand_sb[:], pc, AF.Tanh, scale=1.0 / S)

            t2 = res.tile([128, Mc], F32, name="t2")
            nc.vector.tensor_mul(t2[:], u_sb[:], cand_sb[:])
            outT = res.tile([128, Mc], F32, name="outT")
            nc.vector.tensor_add(outT[:], rh1u[:], t2[:])
            nc.sync.dma_start(d_out[:], outT[:])
            if DBG:
                for k in range(4):
                    nc.sync.dma_start(d_dbg_qbc[:, k * 2048 : (k + 1) * 2048], q_bck[k][:])
                nc.sync.dma_start(d_dbg_cgT[:, 0:Mc], cgT[0][:])
                nc.sync.dma_start(d_dbg_cgT[:, Mc : 2 * Mc], cgT[1][:])
                nc.sync.dma_start(d_dbg_r[:], rh_b[:])
                nc.sync.dma_start(d_dbg_u[:], u_sb[:])
                nc.sync.dma_start(d_dbg_cand[:], cand_sb[:])

    nc.compile()
    return nc


def _get_compiled():
    global _COMPILED
    if _COMPILED is None:
        _COMPILED = _build()
    return _COMPILED


def _pmajor(a, width):
    """[n_tiles*128, width] row-major -> [128, n_tiles*width] partition-major."""
    nt = a.shape[0] // 128
    return np.ascontiguousarray(
        a.reshape(nt, 128, width).transpose(1, 0, 2).reshape(128, nt * width)
    )


def _prep_inputs(x, h, query_vectors, adj, nodes_ind, W_u, b_u, W_r, b_r, W_c, b_c):
    idx = np.asarray(nodes_ind).astype(np.int64)
    f32 = np.float32
    f8 = ml_dtypes.float8_e4m3
    x = np.asarray(x, f32)
    h = np.asarray(h, f32)
    q = np.asarray(query_vectors, f32)
    adj = np.asarray(adj, f32)

    C_dev = np.concatenate([x[:, :128], h], axis=1)  # [N, 256]
    C_pm = np.concatenate(
        [_pmajor(C_dev[:, :128], 128), _pmajor(C_dev[:, 128:], 128)], axis=1
    ).astype(f8)  # [128, 64*128]: C0 block then C1 block
    A_sel = adj[idx]  # [M, N]
    x_sel = x[idx]
    h_sel = h[idx]
    cg_L = A_sel @ x[:, 128]  # [M] leftover mixed feature, on host

    def flat_main(W):
        W = np.asarray(W, f32)
        main = np.concatenate([W[:, :128, :], W[:, 129:, :]], axis=1).reshape(
            QD * 256, O
        )
        return main.reshape(QD * 2, 128, O)  # tile index = d*2 + ihalf

    def order_ru(tiles):
        # (k-chunk, ih, d') order; within a (k, ih) block the 8 d-tiles
        # are consecutive so DR pairs and bf16 singles both slice cleanly
        order = []
        for k in range(4):
            for ih in range(2):
                for dd in range(8):
                    d = 8 * k + dd
                    order.append(d * 2 + ih)
        return tiles[order].reshape(QD * 256, O)

    def order_c(tiles):
        # ihalf-major: all (d, 0) tiles then all (d, 1)
        return np.concatenate([tiles[0::2], tiles[1::2]], axis=0).reshape(
            QD * 256, O
        )

    Wr_pm = _pmajor(order_ru(flat_main(W_r)), O).astype(f8)
    Wu_pm = _pmajor(order_ru(flat_main(W_u)), O).astype(f8)
    Wc_pm = _pmajor(order_c(flat_main(W_c)), O).astype(bfloat16)
    Wt = {g: np.ascontiguousarray(np.asarray(W, f32)[:, 128, :]).astype(bfloat16)
          for g, W in (("r", W_r), ("u", W_u), ("c", W_c))}
    bf_ = {g: (np.asarray(b, f32) * S).astype(bfloat16)
           for g, b in (("r", b_r), ("u", b_u), ("c", b_c))}

    in_maps = []
    for c in range(NC):
        sl = slice(c * Mc, (c + 1) * Mc)
        qT = np.ascontiguousarray(q[sl].T).astype(bfloat16)  # [32, 256]
        AT = np.ascontiguousarray(A_sel[sl].T * np.float32(N))  # [4096, 256]
        SM = np.concatenate(
            [
                qT,
                bf_["r"],
                bf_["u"],
                bf_["c"],
                np.broadcast_to(x_sel[sl, 128] * S, (QD, Mc)).astype(bfloat16),
                np.broadcast_to(cg_L[sl] * S, (QD, Mc)).astype(bfloat16),
                Wt["r"],
                Wt["u"],
                Wt["c"],
            ],
            axis=1,
        )
        # zc0 host-side: ZC[i, d*Mc + n] = x_sel[n, i] * q[n, d] * S
        ZC = (
            x_sel[sl, :128].T[:, None, :] * (q[sl].T[None, :, :] * np.float32(S))
        ).reshape(128, KROWS)
        in_maps.append(
            {
                "AT": _pmajor(AT, Mc).astype(f8),
                "C": C_pm,
                "ZC": ZC.astype(bfloat16),
                "Wr": Wr_pm,
                "Wu": Wu_pm,
                "Wc": Wc_pm,
                "SM": np.ascontiguousarray(SM),
                "qTflat": (qT.reshape(1, KROWS).astype(f32) * (S / N)).astype(bfloat16),
                "hTN": np.ascontiguousarray(h_sel[sl].T * np.float32(N)).astype(bfloat16),
            }
        )
    return in_maps


def run(inputs: dict, trace: bool = False):
    nc = _get_compiled()
    in_maps = _prep_inputs(**inputs)
    res = run_bass_kernel_spmd(nc, in_maps, core_ids=list(range(NC)), trace=trace)
    shards = [res.results[c]["out"].T for c in range(NC)]  # each [256, 128]
    out = np.concatenate(shards, axis=0).astype(np.float32)  # [M, 128]
    return out, res


def kernel(**inputs) -> np.ndarray:
    out, _ = run(inputs, trace=bool(os.environ.get("BASS_KERNEL_TRACE")))
    return out
